# revision 4
# baseline (speedup 1.0000x reference)
"""AttLoRA MoE-routing kernel for 8 Trainium2 NeuronCores — bf16-base version.

Reference computation (per problem nn_AttLoRAModule_85839216378078):
    base  = x @ W_org.T                                    [B,S,OUT]
    q     = x.mean(axis=1) @ Wq.T                          [B,K]
    coef  = softmax(q @ lora_keys.T / sqrt(K))             [B,E]
    h     = x @ lora_down[e]                               [B,S,E,R]
    delta = sum_e coef[b,e] * (h[...,e,:] @ lora_up[e])    [B,S,OUT]
    out   = base + delta * SCALE
Sharding: 8 cores = 4 batches x 2 OUT-halves.  Core c handles batch c//2,
output columns [(c%2)*2048, (c%2+1)*2048).  Each core sees the full x[b], so
the router is computed per core with no collectives.

Numerics/perf: on TRN2 a matmul's issue cost is ~N/2.4GHz for N moving
columns regardless of dtype; fp8-DoubleRow only doubles the contraction per
instruction.  The previous 3-term fp8 residual split therefore cost 48
DR-matmuls per [128,512] output tile where a single bf16 pass costs 32 with
far better accuracy.  This version runs the base matmul in one bf16 pass
(x bf16 stationary, W bf16 moving) and keeps the LoRA path in fp8-DR:
    ps    = x_bf16 @ W_bf16 + tT.T @ lsc                (one PSUM group/tile)
    tT    = fp8((x1 @ (32*ldn)) / 512) = t/16           (phase T, fp8-DR)
    lsc   = fp8(0.25 * coef * (64*lup))                 (coeff folded on-dev)
    ps    = base + delta  (scale 1);  out = bf16(ps)
Router: scores accumulated in one PSUM bank via fp8-DR matmuls against
mk = fp8(65536 * Wq.T@keys.T/(S*sqrt(K))), softmax on-device with the 65536
descale folded into the Exp activation's scale.

SBUF plan (per partition): x bf16 sliding window 3x32K; W bf16 streamed in
[P,8,512] k-pieces (5x8K, two passes over HBM via the s-half outer loop);
x1 fp8 streamed per half-chunk for phase T; ldn 16K; tT 8K; lup/lsc 16K.
Main loop runs 4 concurrent PSUM groups (quad of s-subtiles) accumulating
k-piece columns so W prefetch never stalls the PE.
"""

import math
import os

import numpy as np

import concourse.bacc as bacc
import concourse.mybir as mybir
import concourse.tile as tile
from concourse.bass_utils import run_bass_kernel_spmd

# Problem shapes (hardcoded per contest contract)
B, S, IN, OUT = 4, 2048, 4096, 4096
E, R, K = 8, 64, 128
ER = E * R            # 512
OH = OUT // 2         # 2048 output cols per core
P = 128
IOP = IN // P         # 32 k-subtiles
JP = IOP // 2         # 16 k-pairs (DoubleRow)
NCH = OH // 512       # 4 output column chunks
SC = S // 512         # 4 s-chunks
NQ = 4                # W k-pieces per n-chunk ([P, 8, 512] each)
QW = IOP // NQ        # 8 k-subtiles per W piece

# scale constants
S_LDN = 32.0          # ldn input holds 32*lora_down
S_TT = 1.0 / 512.0    # psum(=32*t) * 1/512 -> tT = t/16
S_LUP = 64.0          # lup input holds 64*lora_up
S_LSC = 0.25          # lsc = 0.25*coef*lup_in -> tT.T@lsc = coef*(t@lup)
S_MK = 65536.0        # mk input holds 65536*mk_true

F32 = mybir.dt.float32
BF16 = mybir.dt.bfloat16
F8 = mybir.dt.float8e4
DR = mybir.MatmulPerfMode.DoubleRow

_NC_CACHE = {}


def _build_nc():
    nc = bacc.Bacc("TRN2", target_bir_lowering=False, debug=False)

    # All inputs pre-laid-out on host to match SBUF tile shapes exactly
    # (partition-major, chunk-contiguous) so every DMA moves large
    # contiguous runs per partition.
    xbc = nc.dram_tensor("xbc", [SC, P, IOP, 512], BF16, kind="ExternalInput")
    x1c = nc.dram_tensor("x1c", [2 * SC, P, IOP, 256], F8, kind="ExternalInput")
    wc = nc.dram_tensor("wc", [NCH, P, IOP, 512], BF16, kind="ExternalInput")
    ldnc = nc.dram_tensor("ldnc", [P, IOP, ER], F8, kind="ExternalInput")
    lupc = nc.dram_tensor("lupc", [NCH, P, ER // P, 512], F8, kind="ExternalInput")
    mkc = nc.dram_tensor("mkc", [P, IOP, 32], F8, kind="ExternalInput")
    cind = nc.dram_tensor("cind", [E, ER], F32, kind="ExternalInput")
    out = nc.dram_tensor("out", [S, OH], BF16, kind="ExternalOutput")

    xbc_ap, x1c_ap, wc_ap, ldnc_ap, lupc_ap, mkc_ap, cind_ap, out_ap = (
        t.ap() for t in (xbc, x1c, wc, ldnc, lupc, mkc, cind, out)
    )

    trace_sim = os.environ.get("KERNEL_SIM_TRACE", "0") == "1"
    with tile.TileContext(nc, trace_sim=trace_sim) as tc:
        with (
            tc.tile_pool(name="xpool", bufs=3) as xpool,
            tc.tile_pool(name="x1pool", bufs=2) as x1pool,
            tc.tile_pool(name="wpool", bufs=5) as wpool,
            tc.tile_pool(name="ldpool", bufs=1) as ldpool,
            tc.tile_pool(name="tpool", bufs=1) as tpool,
            tc.tile_pool(name="lpool", bufs=1) as lpool,
            tc.tile_pool(name="opool", bufs=4) as opool,
            tc.tile_pool(name="rpool", bufs=1) as rpool,
            tc.tile_pool(name="pop", bufs=6, space="PSUM") as pop,
            tc.tile_pool(name="prp", bufs=1, space="PSUM") as prp,
        ):
            # --- small persistent tiles ---
            mk_sb = rpool.tile([P, IOP, 32], F8, tag="mk", name="mk_sb")
            nc.sync.dma_start(mk_sb[:], mkc_ap)
            cind_sb = rpool.tile([E, ER], F32, tag="cind", name="cind_sb")
            nc.gpsimd.dma_start(cind_sb[:], cind_ap)
            ones8 = rpool.tile([E, 1], F32, tag="ones8", name="ones8")
            nc.any.memset(ones8[:], 1.0)
            ones_row = rpool.tile([1, P], F32, tag="ones_row", name="ones_row")
            nc.any.memset(ones_row[:], 1.0)
            coeff_cols = rpool.tile([P, ER // P], F32, tag="coeff", name="coeff_cols")

            # --- streamed ldn (phase T stationary) ---
            ldn_sb = ldpool.tile([P, IOP, ER], F8, tag="ldn", name="ldn_sb")
            nc.scalar.dma_start(ldn_sb[:], ldnc_ap)

            # --- lup chunks early (tiny); scaled into lsc after router ---
            lraw = []
            for n in range(NCH):
                t = lpool.tile([P, ER // P, 512], F8, tag=f"lraw{n}", name=f"lraw_{n}")
                nc.scalar.dma_start(t[:], lupc_ap[n])
                lraw.append(t)

            # --- x bf16 sliding window: c0,c1,c2 up-front; c3 after shalf 0 ---
            xb = [None] * SC

            def load_xb(c):
                t = xpool.tile([P, IOP, 512], BF16, tag="xb", name=f"xb_{c}")
                nc.gpsimd.dma_start(t[:], xbc_ap[c])
                xb[c] = t

            for c in range(SC - 1):
                load_xb(c)

            # --- W bf16 k-pieces, streamed; first n-chunk prefetched early ---
            wsb = {}

            def load_w(sh, n, q):
                t = wpool.tile([P, QW, 512], BF16, tag="w", name=f"w{sh}_{n}_{q}")
                (nc.sync if (n * NQ + q) % 2 == 0 else nc.scalar).dma_start(
                    t[:], wc_ap[n][:, q * QW : (q + 1) * QW, :]
                )
                wsb[(sh, n, q)] = t

            # --- persistent LoRA intermediate ---
            tT = tpool.tile([P, ER // P, S], F8, tag="tT", name="tT")
            pr_t = prp.tile([32, 256], F32, tag="pr", name="pr_t")

            # --- phase T + router projection, per x1 half-chunk as it arrives ---
            for ch in range(2 * SC):
                x1h = x1pool.tile([P, IOP, 256], F8, tag="x1", name=f"x1_{ch}")
                nc.sync.dma_start(x1h[:], x1c_ap[ch])
                for j in range(JP):
                    nc.tensor.matmul(
                        pr_t[:],
                        mk_sb[:, 2 * j : 2 * j + 2, :],
                        x1h[:, 2 * j : 2 * j + 2, :],
                        start=(ch == 0 and j == 0),
                        stop=(ch == 2 * SC - 1 and j == JP - 1),
                        perf_mode=DR,
                    )
                for u in range(ER // P):
                    pt = pop.tile([P, 256], F32, tag="po", name=f"pt_{ch}_{u}")
                    for j in range(JP):
                        nc.tensor.matmul(
                            pt[:],
                            ldn_sb[:, 2 * j : 2 * j + 2, u * P : (u + 1) * P],
                            x1h[:, 2 * j : 2 * j + 2, :],
                            start=(j == 0),
                            stop=(j == JP - 1),
                            perf_mode=DR,
                        )
                    nc.scalar.activation(
                        tT[:, u, ch * 256 : (ch + 1) * 256],
                        pt[:],
                        mybir.ActivationFunctionType.Copy,
                        scale=S_TT,
                    )

            # first n-chunk's W pieces (enqueued after phase T's x1 DMAs so
            # they don't delay the PE-critical router/phase-T path)
            for q in range(NQ):
                load_w(0, 0, q)

            # --- router finalize: softmax over 8 expert scores ---
            scores = rpool.tile([E, 1], F32, tag="scores", name="scores")
            nc.vector.reduce_sum(scores[:], pr_t[:E, :], axis=mybir.AxisListType.X)
            exps = rpool.tile([E, 1], F32, tag="exps", name="exps")
            nc.scalar.activation(
                exps[:],
                scores[:],
                mybir.ActivationFunctionType.Exp,
                scale=1.0 / S_MK,
            )
            psum_s = pop.tile([1, 1], F32, tag="po", name="psum_s")
            nc.tensor.matmul(psum_s[:], exps[:], ones8[:], start=True, stop=True)
            rinv = rpool.tile([1, 1], F32, tag="rinv", name="rinv")
            nc.vector.reciprocal(rinv[:], psum_s[:])
            rb_p = pop.tile([P, 1], F32, tag="po", name="rb_p")
            nc.tensor.matmul(rb_p[:], ones_row[:], rinv[:], start=True, stop=True)
            rb = rpool.tile([P, 1], F32, tag="rb", name="rb")
            nc.vector.tensor_copy(rb[:], rb_p[:])
            cc_un = rpool.tile([P, ER // P], F32, tag="ccun", name="cc_un")
            for u in range(ER // P):
                pcc = pop.tile([P, 1], F32, tag="po", name=f"pcc_{u}")
                nc.tensor.matmul(
                    pcc[:],
                    cind_sb[:, u * P : (u + 1) * P],
                    exps[:],
                    start=True,
                    stop=True,
                )
                nc.vector.tensor_copy(cc_un[:, u : u + 1], pcc[:])
            # coeff_cols = cc_un * (1/sum_exp) * S_LSC
            nc.vector.tensor_scalar(
                coeff_cols[:],
                cc_un[:],
                rb[:],
                S_LSC,
                mybir.AluOpType.mult,
                mybir.AluOpType.mult,
            )

            # --- scaled lup (coeff folded), all chunks resident ---
            lsc = []
            for n in range(NCH):
                t = lpool.tile([P, ER // P, 512], F8, tag=f"lsc{n}", name=f"lsc_{n}")
                nc.vector.tensor_tensor(
                    t[:],
                    lraw[n][:],
                    coeff_cols[:, :, None].to_broadcast((P, ER // P, 512)),
                    mybir.AluOpType.mult,
                )
                lsc.append(t)

            # --- main loop: 2 s-halves x 4 n-chunks x 2 quads x 4 tiles ---
            # Within a (shalf, n): quads of 4 concurrent PSUM groups; each W
            # k-piece is consumed column-wise across the quad so a piece dies
            # quickly and the 5-buffer rotation prefetches ahead stall-free.
            for sh in range(2):
                for n in range(NCH):
                    # prefetch next n-chunk's pieces (or next shalf's n=0)
                    if n + 1 < NCH:
                        for q in range(NQ):
                            load_w(sh, n + 1, q)
                    elif sh == 0:
                        for q in range(NQ):
                            load_w(1, 0, q)
                    if sh == 0 and n == NCH - 1:
                        load_xb(SC - 1)  # c3 into c0's slot for s-half 1
                    for quad in range(2):
                        ms = [sh * 8 + quad * 4 + i for i in range(4)]
                        ps = {}
                        for m in ms:
                            ps[m] = pop.tile(
                                [P, 512], F32, tag="po", name=f"ps_{sh}_{n}_{m}"
                            )
                        for q in range(NQ):
                            wt = wsb[(sh, n, q)]
                            for m in ms:
                                c, mm = m // 4, m % 4
                                xsl = slice(mm * 128, (mm + 1) * 128)
                                for i in range(QW):
                                    nc.tensor.matmul(
                                        ps[m][:],
                                        xb[c][:, q * QW + i, xsl],
                                        wt[:, i, :],
                                        start=(q == 0 and i == 0),
                                        stop=False,
                                    )
                        for m in ms:
                            for uu in range(ER // P // 2):
                                nc.tensor.matmul(
                                    ps[m][:],
                                    tT[:, 2 * uu : 2 * uu + 2, m * P : (m + 1) * P],
                                    lsc[n][:, 2 * uu : 2 * uu + 2, :],
                                    start=False,
                                    stop=(uu == ER // P // 2 - 1),
                                    perf_mode=DR,
                                )
                            ost = opool.tile(
                                [P, 512], BF16, tag="ost", name=f"os_{sh}_{n}_{m}"
                            )
                            nc.vector.tensor_copy(ost[:], ps[m][:])
                            (nc.sync if m % 2 == 0 else nc.scalar).dma_start(
                                out_ap[m * P : (m + 1) * P, n * 512 : (n + 1) * 512],
                                ost[:],
                            )

    nc.compile()
    return nc


def _f8(a):
    import ml_dtypes

    return np.asarray(a, dtype=np.float32).astype(ml_dtypes.float8_e4m3)


def _bf16(a):
    import ml_dtypes

    return np.asarray(a, dtype=np.float32).astype(ml_dtypes.bfloat16)


def _prep_core_inputs(x, W_org, lora_down, lora_up, lora_keys, Wq):
    """Host-side layout/scale prep shared across cores; returns per-core maps."""
    xT = [np.ascontiguousarray(np.asarray(x[b]).T) for b in range(B)]  # [IN,S]
    wT = np.ascontiguousarray(np.asarray(W_org, np.float32).T)         # [IN,OUT]
    ldn = np.ascontiguousarray(
        np.asarray(lora_down, np.float32).transpose(1, 0, 2).reshape(IN, ER)
    )
    lup = np.ascontiguousarray(np.asarray(lora_up, np.float32).reshape(ER, OUT))
    mk = (np.asarray(Wq, np.float32).T @ np.asarray(lora_keys, np.float32).T) / (
        S * math.sqrt(K)
    )

    def iomaj(a, ncols):  # [IN, C] -> [C//ncols, P, IOP, ncols]
        return np.ascontiguousarray(
            a.reshape(IOP, P, a.shape[1] // ncols, ncols).transpose(2, 1, 0, 3)
        )

    # x: bf16 (base stationary) + fp8 half-chunks (phase T / router moving)
    xb_l, x1_l = [], []
    for b in range(B):
        xb_l.append(iomaj(_bf16(xT[b]), 512))
        x1_l.append(iomaj(_f8(xT[b]), 256))

    # W: bf16, per OH half
    w_l = [iomaj(_bf16(wT[:, h * OH : (h + 1) * OH]), 512) for h in range(2)]

    ldnc = np.ascontiguousarray(
        _f8(S_LDN * ldn).reshape(IOP, P, ER).transpose(1, 0, 2)
    )
    lup8 = _f8(S_LUP * lup)
    lupc_l = [
        np.ascontiguousarray(
            lup8[:, h * OH : (h + 1) * OH]
            .reshape(ER // P, P, NCH, 512)
            .transpose(2, 1, 0, 3)
        )
        for h in range(2)
    ]
    mk_pad = np.zeros((IN, 32), np.float32)
    mk_pad[:, :E] = S_MK * mk
    mkc = np.ascontiguousarray(_f8(mk_pad).reshape(IOP, P, 32).transpose(1, 0, 2))
    cind_np = np.repeat(np.eye(E, dtype=np.float32), R, axis=1)

    in_maps = []
    for c in range(8):
        b, h = c // 2, c % 2
        in_maps.append(
            {
                "xbc": xb_l[b],
                "x1c": x1_l[b],
                "wc": w_l[h],
                "ldnc": ldnc,
                "lupc": lupc_l[h],
                "mkc": mkc,
                "cind": cind_np,
            }
        )
    return in_maps


def kernel(x, W_org, lora_down, lora_up, lora_keys, Wq):
    in_maps = _prep_core_inputs(x, W_org, lora_down, lora_up, lora_keys, Wq)

    if "nc" not in _NC_CACHE:
        _NC_CACHE["nc"] = _build_nc()
    nc = _NC_CACHE["nc"]

    res = run_bass_kernel_spmd(nc, in_maps, core_ids=list(range(8)), trace=False)
    _NC_CACHE["last_result"] = res
    _NC_CACHE["last_in_maps"] = in_maps

    outp = np.empty((B, S, OUT), dtype=np.float32)
    for c in range(8):
        b, h = c // 2, c % 2
        outp[b, :, h * OH : (h + 1) * OH] = res.results[c]["out"].astype(np.float32)
    return outp


# revision 24
# speedup vs baseline: 1.0162x; 1.0162x over previous
"""AttLoRA MoE-routing kernel for 8 Trainium2 NeuronCores — bf16-base version.

Reference computation (per problem nn_AttLoRAModule_85839216378078):
    base  = x @ W_org.T                                    [B,S,OUT]
    q     = x.mean(axis=1) @ Wq.T                          [B,K]
    coef  = softmax(q @ lora_keys.T / sqrt(K))             [B,E]
    h     = x @ lora_down[e]                               [B,S,E,R]
    delta = sum_e coef[b,e] * (h[...,e,:] @ lora_up[e])    [B,S,OUT]
    out   = base + delta * SCALE
Sharding: 8 cores = 4 batches x 2 OUT-halves.  Core c handles batch c//2,
output columns [(c%2)*2048, (c%2+1)*2048).  Each core sees the full x[b], so
the router is computed per core with no collectives.

Numerics/perf: on TRN2 a matmul's issue cost is ~N/2.4GHz for N moving
columns regardless of dtype; fp8-DoubleRow only doubles the contraction per
instruction.  The previous 3-term fp8 residual split therefore cost 48
DR-matmuls per [128,512] output tile where a single bf16 pass costs 32 with
far better accuracy.  This version runs the base matmul in one bf16 pass
(x bf16 stationary, W bf16 moving) and keeps the LoRA path in fp8-DR:
    ps    = x_bf16 @ W_bf16 + tT.T @ lsc                (one PSUM group/tile)
    tT    = fp8((x1 @ (32*ldn)) / 512) = t/16           (phase T, fp8-DR)
    lsc   = fp8(0.25 * coef * (64*lup))                 (coeff folded on-dev)
    ps    = base + delta  (scale 1);  out = bf16(ps)
Router: scores accumulated in one PSUM bank via fp8-DR matmuls against
mk = fp8(65536 * Wq.T@keys.T/(S*sqrt(K))), softmax on-device with the 65536
descale folded into the Exp activation's scale.

SBUF plan (per partition): x bf16 sliding window 3x32K; W bf16 streamed in
[P,8,512] k-pieces (5x8K, two passes over HBM via the s-half outer loop);
x1 fp8 streamed per half-chunk for phase T; ldn 16K; tT 8K; lup/lsc 16K.
Main loop runs 4 concurrent PSUM groups (quad of s-subtiles) accumulating
k-piece columns so W prefetch never stalls the PE.
"""

import math
import os

import numpy as np

import concourse.bacc as bacc
import concourse.mybir as mybir
import concourse.tile as tile
from concourse.bass_utils import run_bass_kernel_spmd

# Problem shapes (hardcoded per contest contract)
B, S, IN, OUT = 4, 2048, 4096, 4096
E, R, K = 8, 64, 128
ER = E * R            # 512
OH = OUT // 2         # 2048 output cols per core
P = 128
IOP = IN // P         # 32 k-subtiles
JP = IOP // 2         # 16 k-pairs (DoubleRow)
NCH = OH // 512       # 4 output column chunks
SC = S // 512         # 4 s-chunks
NQ = 2                # W k-pieces per n-chunk ([P, 16, 512] each)
QW = IOP // NQ        # 16 k-subtiles per W piece

# scale constants
S_LDN = 32.0          # ldn input holds 32*lora_down
S_TT = 1.0 / 512.0    # psum(=32*t) * 1/512 -> tT = t/16
S_LUP = 64.0          # lup input holds 64*lora_up
S_LSC = 0.25          # lsc = 0.25*coef*lup_in -> tT.T@lsc = coef*(t@lup)
S_MK = 65536.0        # mk input holds 65536*mk_true

F32 = mybir.dt.float32
BF16 = mybir.dt.bfloat16
F8 = mybir.dt.float8e4
DR = mybir.MatmulPerfMode.DoubleRow

_NC_CACHE = {}


def _build_nc():
    nc = bacc.Bacc("TRN2", target_bir_lowering=False, debug=False)

    # All inputs pre-laid-out on host to match SBUF tile shapes exactly
    # (partition-major, chunk-contiguous) so every DMA moves large
    # contiguous runs per partition.
    xbc = nc.dram_tensor("xbc", [SC, 2, P, IOP, 256], BF16, kind="ExternalInput")
    wc = nc.dram_tensor("wc", [NCH, P, IOP, 512], BF16, kind="ExternalInput")
    ldnc = nc.dram_tensor("ldnc", [P, IOP, ER], F8, kind="ExternalInput")
    lupc = nc.dram_tensor("lupc", [NCH, P, ER // P, 512], F8, kind="ExternalInput")
    mkc = nc.dram_tensor("mkc", [P, IOP, 32], F8, kind="ExternalInput")
    cind = nc.dram_tensor("cind", [E, ER], F32, kind="ExternalInput")
    out = nc.dram_tensor("out", [S, OH], BF16, kind="ExternalOutput")

    xbc_ap, wc_ap, ldnc_ap, lupc_ap, mkc_ap, cind_ap, out_ap = (
        t.ap() for t in (xbc, wc, ldnc, lupc, mkc, cind, out)
    )

    trace_sim = os.environ.get("KERNEL_SIM_TRACE", "0") == "1"
    with tile.TileContext(nc, trace_sim=trace_sim) as tc:
        with (
            tc.tile_pool(name="xpool", bufs=3) as xpool,
            tc.tile_pool(name="x8pool", bufs=2) as x8pool,
            tc.tile_pool(name="wpool", bufs=3) as wpool,
            tc.tile_pool(name="ldpool", bufs=1) as ldpool,
            tc.tile_pool(name="tpool", bufs=1) as tpool,
            tc.tile_pool(name="lpool", bufs=1) as lpool,
            tc.tile_pool(name="opool", bufs=2) as opool,
            tc.tile_pool(name="rpool", bufs=1) as rpool,
            tc.tile_pool(name="pop", bufs=5, space="PSUM") as pop,
            tc.tile_pool(name="rlp", bufs=2, space="PSUM") as rlp,
            tc.tile_pool(name="prp", bufs=1, space="PSUM") as prp,
        ):
            # --- small persistent tiles ---
            mk_sb = rpool.tile([P, IOP, 32], F8, tag="mk", name="mk_sb")
            nc.sync.dma_start(mk_sb[:], mkc_ap)
            ones8 = rpool.tile([E, 1], F32, tag="ones8", name="ones8")
            nc.any.memset(ones8[:], 1.0)
            ones_row = rpool.tile([1, P], F32, tag="ones_row", name="ones_row")
            nc.any.memset(ones_row[:], 1.0)
            coeff_cols = rpool.tile([P, ER // P], F32, tag="coeff", name="coeff_cols")

            # --- streamed ldn (phase T stationary) ---
            ldn_sb = ldpool.tile([P, IOP, ER], F8, tag="ldn", name="ldn_sb")
            nc.scalar.dma_start(ldn_sb[:], ldnc_ap)

            # --- lup chunks early (tiny); scaled into lsc after router ---
            lraw = []
            for n in range(NCH):
                t = lpool.tile([P, ER // P, 512], F8, tag=f"lraw{n}", name=f"lraw_{n}")
                nc.scalar.dma_start(t[:], lupc_ap[n])
                lraw.append(t)

            # --- x bf16 sliding window: c0,c1,c2 up-front; c3 after shalf 0.
            # Loaded in s-half pieces so phase T can start on the first 2 MiB.
            xb = [None] * SC

            def load_xb(c):
                t = xpool.tile([P, 2, IOP, 256], BF16, tag="xb", name=f"xb_{c}")
                nc.gpsimd.dma_start(t[:, 0], xbc_ap[c][0])
                nc.gpsimd.dma_start(t[:, 1], xbc_ap[c][1])
                xb[c] = t

            # Phase T consumes chunks in order c3,c0,c1,c2 so that c2 can
            # reuse c3's slot (c3 is re-loaded for s-half 1 of the main loop).
            # cind rides gpsimd behind the PE-critical first halves.
            load_xb(3)
            cind_sb = rpool.tile([E, ER], F32, tag="cind", name="cind_sb")
            nc.gpsimd.dma_start(cind_sb[:], cind_ap)
            for c in (0, 1):
                load_xb(c)

            # --- W bf16 k-pieces, streamed; first n-chunk prefetched early ---
            wsb = {}

            def load_w(sh, n, q, eng=None):
                t = wpool.tile([P, QW, 512], BF16, tag="w", name=f"w{sh}_{n}_{q}")
                if eng is None:
                    eng = nc.sync if (n * NQ + q) % 2 == 0 else nc.scalar
                eng.dma_start(t[:], wc_ap[n][:, q * QW : (q + 1) * QW, :])
                wsb[(sh, n, q)] = t

            # --- persistent LoRA intermediate ---
            tT = tpool.tile([P, ER // P, S], F8, tag="tT", name="tT")
            pr_t = prp.tile([32, 256], F32, tag="pr", name="pr_t")

            # --- phase T + router projection, per s-half-chunk as x arrives.
            # The fp8 x for the DR matmuls is cast on-device from the bf16 x
            # (saves HBM traffic and decouples phase T from DMA).
            for ci, c in enumerate((3, 0, 1, 2)):
                for hh in range(2):
                    first = ci == 0 and hh == 0
                    last = ci == SC - 1 and hh == 1
                    so = (2 * c + hh) * 256  # s-offset of this half-chunk
                    x1h = x8pool.tile(
                        [P, IOP, 256], F8, tag="x8", name=f"x8_{c}_{hh}"
                    )
                    nc.vector.tensor_copy(x1h[:], xb[c][:, hh])
                    for j in range(JP):
                        nc.tensor.matmul(
                            pr_t[:],
                            mk_sb[:, 2 * j : 2 * j + 2, :],
                            x1h[:, 2 * j : 2 * j + 2, :],
                            start=(first and j == 0),
                            stop=(last and j == JP - 1),
                            perf_mode=DR,
                        )
                    for u in range(ER // P):
                        pt = pop.tile([P, 256], F32, tag="po", name=f"pt_{c}_{hh}_{u}")
                        for j in range(JP):
                            nc.tensor.matmul(
                                pt[:],
                                ldn_sb[:, 2 * j : 2 * j + 2, u * P : (u + 1) * P],
                                x1h[:, 2 * j : 2 * j + 2, :],
                                start=(j == 0),
                                stop=(j == JP - 1),
                                perf_mode=DR,
                            )
                        nc.scalar.activation(
                            tT[:, u, so : so + 256],
                            pt[:],
                            mybir.ActivationFunctionType.Copy,
                            scale=S_TT,
                        )
                if c == 3:
                    load_xb(2)  # reuses c3's slot once its casts are done

            # first n-chunk's W pieces ride gpsimd behind the xb loads so the
            # front's PE-critical DMAs (xb c3h0, ldn) get the bandwidth
            for q in range(NQ):
                load_w(0, 0, q, eng=nc.gpsimd)

            # --- first quad's base matmuls: keep the PE busy while the
            # router-finalize DVE/ACT chain below runs ---
            def quad_base(sh, n, quad):
                ms = [sh * 8 + quad * 4 + i for i in range(4)]
                ps = {}
                for m in ms:
                    ps[m] = pop.tile(
                        [P, 512], F32, tag="po", name=f"ps_{sh}_{n}_{m}"
                    )
                for q in range(NQ):
                    wt = wsb[(sh, n, q)]
                    for m in ms:
                        c, mm = m // 4, m % 4
                        hh, oo = mm // 2, mm % 2
                        xsl = slice(oo * 128, (oo + 1) * 128)
                        for i in range(QW):
                            nc.tensor.matmul(
                                ps[m][:],
                                xb[c][:, hh, q * QW + i, xsl],
                                wt[:, i, :],
                                start=(q == 0 and i == 0),
                                stop=False,
                            )
                return ms, ps

            def quad_lora(sh, n, ms, ps):
                for m in ms:
                    for uu in range(ER // P // 2):
                        nc.tensor.matmul(
                            ps[m][:],
                            tT[:, 2 * uu : 2 * uu + 2, m * P : (m + 1) * P],
                            lsc[n][:, 2 * uu : 2 * uu + 2, :],
                            start=False,
                            stop=(uu == ER // P // 2 - 1),
                            perf_mode=DR,
                        )
                    ost = opool.tile(
                        [P, 512], BF16, tag="ost", name=f"os_{sh}_{n}_{m}"
                    )
                    nc.vector.tensor_copy(ost[:], ps[m][:])
                    (nc.sync if m % 2 == 0 else nc.scalar).dma_start(
                        out_ap[m * P : (m + 1) * P, n * 512 : (n + 1) * 512],
                        ost[:],
                    )

            first_quad = quad_base(0, 0, 0)

            # --- router finalize: softmax over 8 expert scores ---
            scores = rpool.tile([E, 1], F32, tag="scores", name="scores")
            nc.vector.reduce_sum(scores[:], pr_t[:E, :], axis=mybir.AxisListType.X)
            exps = rpool.tile([E, 1], F32, tag="exps", name="exps")
            nc.scalar.activation(
                exps[:],
                scores[:],
                mybir.ActivationFunctionType.Exp,
                scale=1.0 / S_MK,
            )
            psum_s = rlp.tile([1, 1], F32, tag="rl", name="psum_s")
            nc.tensor.matmul(psum_s[:], exps[:], ones8[:], start=True, stop=True)
            rinv = rpool.tile([1, 1], F32, tag="rinv", name="rinv")
            nc.vector.reciprocal(rinv[:], psum_s[:])
            rb_p = rlp.tile([P, 1], F32, tag="rl", name="rb_p")
            nc.tensor.matmul(rb_p[:], ones_row[:], rinv[:], start=True, stop=True)
            rb = rpool.tile([P, 1], F32, tag="rb", name="rb")
            nc.vector.tensor_copy(rb[:], rb_p[:])
            cc_un = rpool.tile([P, ER // P], F32, tag="ccun", name="cc_un")
            for u in range(ER // P):
                pcc = rlp.tile([P, 1], F32, tag="rl", name=f"pcc_{u}")
                nc.tensor.matmul(
                    pcc[:],
                    cind_sb[:, u * P : (u + 1) * P],
                    exps[:],
                    start=True,
                    stop=True,
                )
                nc.vector.tensor_copy(cc_un[:, u : u + 1], pcc[:])
            # coeff_cols = cc_un * (1/sum_exp) * S_LSC
            nc.vector.tensor_scalar(
                coeff_cols[:],
                cc_un[:],
                rb[:],
                S_LSC,
                mybir.AluOpType.mult,
                mybir.AluOpType.mult,
            )

            # --- scaled lup (coeff folded), all chunks resident ---
            lsc = []
            for n in range(NCH):
                t = lpool.tile([P, ER // P, 512], F8, tag=f"lsc{n}", name=f"lsc_{n}")
                nc.vector.tensor_tensor(
                    t[:],
                    lraw[n][:],
                    coeff_cols[:, :, None].to_broadcast((P, ER // P, 512)),
                    mybir.AluOpType.mult,
                )
                lsc.append(t)

            # --- main loop: 2 s-halves x 4 n-chunks x 2 quads x 4 tiles ---
            # Within a (shalf, n): quads of 4 concurrent PSUM groups; W
            # k-pieces are consumed column-wise across the quad so a piece
            # dies quickly and the 3-buffer rotation prefetches stall-free.
            # The first quad's base matmuls were emitted above (they cover
            # the router-finalize chain); its lora/drain runs here first.
            quad_lora(0, 0, *first_quad)
            for sh in range(2):
                for n in range(NCH):
                    # prefetch next n-chunk's pieces (or next shalf's n=0)
                    if n + 1 < NCH:
                        for q in range(NQ):
                            load_w(sh, n + 1, q)
                    elif sh == 0:
                        for q in range(NQ):
                            load_w(1, 0, q)
                    if sh == 0 and n == NCH - 1:
                        load_xb(SC - 1)  # c3 into c0's slot for s-half 1
                    for quad in range(2):
                        if sh == 0 and n == 0 and quad == 0:
                            continue
                        ms, ps = quad_base(sh, n, quad)
                        quad_lora(sh, n, ms, ps)

    nc.compile()
    return nc


def _f8(a):
    import ml_dtypes

    return np.asarray(a, dtype=np.float32).astype(ml_dtypes.float8_e4m3)


def _bf16(a):
    import ml_dtypes

    return np.asarray(a, dtype=np.float32).astype(ml_dtypes.bfloat16)


def _prep_core_inputs(x, W_org, lora_down, lora_up, lora_keys, Wq):
    """Host-side layout/scale prep shared across cores; returns per-core maps."""
    xT = [np.ascontiguousarray(np.asarray(x[b]).T) for b in range(B)]  # [IN,S]
    wT = np.ascontiguousarray(np.asarray(W_org, np.float32).T)         # [IN,OUT]
    ldn = np.ascontiguousarray(
        np.asarray(lora_down, np.float32).transpose(1, 0, 2).reshape(IN, ER)
    )
    lup = np.ascontiguousarray(np.asarray(lora_up, np.float32).reshape(ER, OUT))
    mk = (np.asarray(Wq, np.float32).T @ np.asarray(lora_keys, np.float32).T) / (
        S * math.sqrt(K)
    )

    def iomaj(a, ncols):  # [IN, C] -> [C//ncols, P, IOP, ncols]
        return np.ascontiguousarray(
            a.reshape(IOP, P, a.shape[1] // ncols, ncols).transpose(2, 1, 0, 3)
        )

    # x: bf16 in s-half-chunk pieces [SC, 2, P, IOP, 256] (the fp8 copy for
    # phase T is cast on-device)
    xb_l = [
        iomaj(_bf16(xT[b]), 256).reshape(SC, 2, P, IOP, 256) for b in range(B)
    ]

    # W: bf16, per OH half
    w_l = [iomaj(_bf16(wT[:, h * OH : (h + 1) * OH]), 512) for h in range(2)]

    ldnc = np.ascontiguousarray(
        _f8(S_LDN * ldn).reshape(IOP, P, ER).transpose(1, 0, 2)
    )
    lup8 = _f8(S_LUP * lup)
    lupc_l = [
        np.ascontiguousarray(
            lup8[:, h * OH : (h + 1) * OH]
            .reshape(ER // P, P, NCH, 512)
            .transpose(2, 1, 0, 3)
        )
        for h in range(2)
    ]
    mk_pad = np.zeros((IN, 32), np.float32)
    mk_pad[:, :E] = S_MK * mk
    mkc = np.ascontiguousarray(_f8(mk_pad).reshape(IOP, P, 32).transpose(1, 0, 2))
    cind_np = np.repeat(np.eye(E, dtype=np.float32), R, axis=1)

    in_maps = []
    for c in range(8):
        b, h = c // 2, c % 2
        in_maps.append(
            {
                "xbc": xb_l[b],
                "wc": w_l[h],
                "ldnc": ldnc,
                "lupc": lupc_l[h],
                "mkc": mkc,
                "cind": cind_np,
            }
        )
    return in_maps


def kernel(x, W_org, lora_down, lora_up, lora_keys, Wq):
    in_maps = _prep_core_inputs(x, W_org, lora_down, lora_up, lora_keys, Wq)

    if "nc" not in _NC_CACHE:
        _NC_CACHE["nc"] = _build_nc()
    nc = _NC_CACHE["nc"]

    res = run_bass_kernel_spmd(nc, in_maps, core_ids=list(range(8)), trace=False)
    _NC_CACHE["last_result"] = res
    _NC_CACHE["last_in_maps"] = in_maps

    outp = np.empty((B, S, OUT), dtype=np.float32)
    for c in range(8):
        b, h = c // 2, c % 2
        outp[b, :, h * OH : (h + 1) * OH] = res.results[c]["out"].astype(np.float32)
    return outp
